# revision 24
# baseline (speedup 1.0000x reference)
"""DepthAttentionResidual Trainium2 kernel.

Computation (see reference):
    ms      = mean(history^2, axis=-1)                      # [S,B,T]
    logits  = dot(query*rms_weight, history) * rsqrt(ms+eps)
    w       = softmax(logits, axis=S)
    out     = sum_s w[s] * history[s]                        # [B,T,D]

Sharding: data-parallel over (B=4) x (T halves) = 8 cores. Each core gets
hist [S=16, Tc=1024, D=1024] (64 MiB) and produces out [1024, 1024].

Per-core layout: partition p = s*8 + t' (S=16 depths x 8 t-blocks), D on
the free axis. A supertile is 128 t; slice g holds t_local = t'*16 + g,
so one slice is [128 partitions, 1024] and a supertile loads with
full-width 4 KiB-per-partition DMA descriptors (the 128-partition SBUF
port rule makes this ~6x faster than narrow-partition DMAs).
  - sum(h^2) over D: ScalarE activation(Square, accum_out) (last slice
    on VectorE to balance engines)
  - dot(q*w, h) over D: VectorE affine_mul_reduce (tensor_tensor_reduce
    crashes this runtime)
  - softmax over S: sum over s-partition-subgroups via 0/1-mask fp32
    matmuls (exact); exp on ScalarE; reciprocals on VectorE
  - depth mix: per D-half, 16 accumulating fp32r matmuls (1 cycle/row)
    with block-expanded masked weights built on GpSimd (w2[p, c] =
    softmax weight iff c == t_local(p, g)), filling all 128 t rows of a
    PSUM bank; PSUM -> SBUF on VectorE; one 512 KiB DMA per supertile.
Input DMAs ride the SP HWDGE ring; constants + output DMAs ride the
ScalarE ring so the history stream never stalls. The last supertile is
split 64/64 to shorten the serial tail. fp32r rounds operands to ~13
mantissa bits -> ~2e-4 relative output error.

Reads history exactly once (~68 MiB DMA per core): DMA-bound at ~305 us
vs a ~235 us descriptor-rate floor.
"""
import numpy as np

import concourse.bass as bass
import concourse.bacc as bacc
import concourse.tile as tile
from concourse import mybir
from concourse import bass_utils

N_CORES = 8
S = 16
B = 4
T = 2048
D = 1024
EPS = 1e-5

TC = T // 2          # t positions per core
TG = 8               # t-blocks per partition set (S * TG = 128 partitions)
GROUPS = 16          # stat slices per supertile (one t per partition each)
J = 4                # consecutive t rows per DMA slice (descriptor = J*4KiB)
NDMA = GROUPS // J   # input DMAs per supertile
TS = TG * GROUPS     # t per supertile = 128
N_SUPER = TC // TS   # supertiles per core = 8
F32 = mybir.dt.float32
F32R = mybir.dt.float32r


def _build_program():
    nc = bacc.Bacc("TRN2", target_bir_lowering=False, debug=False,
                   enable_asserts=True, num_devices=N_CORES)

    hist = nc.dram_tensor("hist", [S, TC, D], F32R, kind="ExternalInput").ap()
    query = nc.dram_tensor("query", [D], F32, kind="ExternalInput").ap()
    rmsw = nc.dram_tensor("rms_weight", [D], F32, kind="ExternalInput").ap()
    mask_d = nc.dram_tensor("mask", [128, TG], F32, kind="ExternalInput").ap()
    maskt_d = nc.dram_tensor("maskT", [TG, 128], F32, kind="ExternalInput").ap()
    maskf_d = nc.dram_tensor("maskF", [128, GROUPS, 128], F32,
                             kind="ExternalInput").ap()
    maskf8_d = nc.dram_tensor("maskF8", [128, GROUPS // 2, 128 // 2], F32,
                              kind="ExternalInput").ap()
    out = nc.dram_tensor("out", [TC, D], F32, kind="ExternalOutput").ap()

    with tile.TileContext(nc) as tc:
        with (
            tc.tile_pool(name="singles", bufs=1) as singles,
            tc.tile_pool(name="hsup", bufs=2) as hpool,
            tc.tile_pool(name="stats", bufs=2) as stats,
            tc.tile_pool(name="w2", bufs=3) as w2pool,
            tc.tile_pool(name="outp", bufs=2) as outpool,
            tc.tile_pool(name="ps_stats", bufs=2, space="PSUM") as ps_stats,
            tc.tile_pool(name="ps_mix", bufs=2, space="PSUM") as ps_mix,
        ):
            # ---- constants (DMAs emitted after the first supertile's so
            # the history stream starts immediately) ------------------------
            qw = singles.tile([128, D], F32)
            wb = singles.tile([128, D], F32)
            mask = singles.tile([128, TG], F32)
            maskT = singles.tile([TG, 128], F32)
            maskF = singles.tile([128, GROUPS, 128], F32)
            maskF8 = singles.tile([128, GROUPS // 2, 128 // 2], F32)
            epst = singles.tile([128, 1], F32)
            dummy_a = singles.tile([128, 1], F32)
            dummy_v = singles.tile([128, 1], F32)

            def emit_init():
                # constants ride the ScalarE HWDGE queue so they don't
                # delay the history stream on the SP queue
                nc.scalar.dma_start(
                    out=qw[:],
                    in_=bass.AP(tensor=query.tensor, offset=0,
                                ap=[[0, 128], [1, D]]),
                )
                nc.scalar.dma_start(
                    out=wb[:],
                    in_=bass.AP(tensor=rmsw.tensor, offset=0,
                                ap=[[0, 128], [1, D]]),
                )
                nc.vector.tensor_mul(qw[:], qw[:], wb[:])  # query * rms_weight
                nc.scalar.dma_start(out=mask[:], in_=mask_d)
                nc.scalar.dma_start(out=maskT[:], in_=maskt_d)
                nc.scalar.dma_start(out=maskF[:], in_=maskf_d)
                nc.scalar.dma_start(out=maskF8[:], in_=maskf8_d)
                nc.vector.memset(epst[:], EPS)

            # ---- main loop over supertiles --------------------------------
            # last 128-t supertile is split in two 64-t halves to shorten
            # the serial tail after the final DMA
            emit_init()
            schedule = [(k * TS, GROUPS) for k in range(N_SUPER - 1)]
            schedule += [((N_SUPER - 1) * TS, GROUPS // 2),
                         ((N_SUPER - 1) * TS + TS // 2, GROUPS // 2)]
            for k, (t0, groups) in enumerate(schedule):
                ts_k = TG * groups
                ndma = groups // J

                # load [S, 128t, D] as partitions (s, t') x free (g, d)
                # where t_local = t' * GROUPS + g: one full-width DMA with
                # 64 KiB contiguous per partition (128-partition P1 rule)
                half = max(ndma // 2, 1)
                hsupA = hpool.tile([128, half, J, D], F32R, tag="hsupA",
                                   name="hsupA")
                hsupB = hpool.tile([128, ndma - half, J, D], F32R,
                                   tag="hsupB", name="hsupB", bufs=3)

                def hslice(g):
                    gd, j = g // J, g % J
                    tile_ = hsupA if gd < half else hsupB
                    return tile_[:, gd - half if gd >= half else gd, j, :]

                srcv = hist[:, t0:t0 + ts_k, :].rearrange(
                    "s (t gd j) d -> s t gd (j d)", t=TG, gd=ndma)
                for gd in range(ndma):
                    tile_ = hsupA if gd < half else hsupB
                    nc.sync.dma_start(
                        out=tile_[:, gd - half if gd >= half else gd, :, :]
                        .rearrange("p j d -> p (j d)"),
                        in_=srcv[:, :, gd, :])


                ss = stats.tile([128, groups], F32, tag="ss")
                dot = stats.tile([128, groups], F32, tag="dot")
                for g in range(groups):
                    h_g = hslice(g).bitcast(F32)
                    if g < groups - 1:
                        nc.scalar.activation(
                            out=dummy_a.broadcast_to([128, D]),
                            in_=h_g,
                            func=mybir.ActivationFunctionType.Square,
                            accum_out=ss[:, g:g + 1],
                        )
                    else:
                        # last two sumsq on VectorE: keeps ScalarE below the
                        # DMA pace
                        nc.vector.affine_mul_reduce(
                            out=dummy_v.broadcast_to([128, D]),
                            accum_out=ss[:, g:g + 1],
                            in0=h_g, in1=h_g, scale=1.0, bias=0.0,
                        )
                    nc.vector.affine_mul_reduce(
                        out=dummy_v.broadcast_to([128, D]),
                        accum_out=dot[:, g:g + 1],
                        in0=h_g,
                        in1=qw[:],
                        scale=1.0,
                        bias=0.0,
                    )

                # rstd = 1/sqrt(ss/D + eps); logits = dot * rstd; e = exp
                sd = stats.tile([128, groups], F32, tag="sd")
                nc.scalar.activation(
                    out=sd[:], in_=ss[:],
                    func=mybir.ActivationFunctionType.Sqrt,
                    bias=epst[:], scale=1.0 / D,
                )
                rstd = stats.tile([128, groups], F32, tag="rstd")
                nc.vector.reciprocal(out=rstd[:], in_=sd[:])
                logit = stats.tile([128, groups], F32, tag="logit")
                nc.vector.tensor_mul(logit[:], dot[:], rstd[:])
                e = stats.tile([128, groups], F32, tag="e")
                nc.scalar.activation(
                    out=e[:], in_=logit[:],
                    func=mybir.ActivationFunctionType.Exp,
                )

                # sumexp over s: [8t', G] = mask^T @ e (exact fp32 matmul)
                se_ps = ps_stats.tile([TG, groups], F32, tag="se")
                nc.tensor.matmul(out=se_ps[:], lhsT=mask[:], rhs=e[:],
                                 start=True, stop=True)
                rse = stats.tile([TG, groups], F32, tag="rse")
                nc.vector.reciprocal(out=rse[:], in_=se_ps[:])
                # broadcast rse back to (s,t') partitions: maskT^T @ rse
                rseb_ps = ps_stats.tile([128, groups], F32, tag="rseb")
                nc.tensor.matmul(out=rseb_ps[:], lhsT=maskT[:], rhs=rse[:],
                                 start=True, stop=True)
                rseb = stats.tile([128, groups], F32, tag="rsebs")
                nc.vector.tensor_copy(out=rseb[:], in_=rseb_ps[:])

                # depth mix: accumulate 16 masked-weight matmuls per D chunk
                m_ps = [ps_mix.tile([TG * groups, 512], F32, tag="m", name=f"m{c}")
                        for c in range(2)]
                for g in range(groups):
                    w2 = w2pool.tile([128, TG * groups], F32R, tag="w2")
                    nc.gpsimd.tensor_scalar(
                        out=w2[:],
                        in0=(maskF[:, g, :] if groups == GROUPS
                             else maskF8[:, g, :]),
                        scalar1=e[:, g:g + 1],
                        scalar2=rseb[:, g:g + 1],
                        op0=mybir.AluOpType.mult,
                        op1=mybir.AluOpType.mult,
                    )
                    for c in range(2):
                        nc.tensor.matmul(
                            out=m_ps[c][:],
                            lhsT=w2[:],
                            rhs=hslice(g)[:, c * 512:(c + 1) * 512],
                            start=(g == 0),
                            stop=(g == groups - 1),
                        )

                ot = outpool.tile([TG * groups, D], F32, tag="ot")
                for c in range(2):
                    nc.vector.tensor_copy(out=ot[:, c * 512:(c + 1) * 512],
                                          in_=m_ps[c][:])
                nc.scalar.dma_start(out=out[t0:t0 + ts_k, :], in_=ot[:])

    nc.compile()
    return nc


_NC = None


def _get_program():
    global _NC
    if _NC is None:
        _NC = _build_program()
    return _NC


def _make_masks():
    # partition p = s*TG + t'; group slice g holds t_local = t'*GROUPS + g
    p = np.arange(128)
    mask = (p[:, None] % TG == np.arange(TG)[None, :]).astype(np.float32)
    maskF = np.zeros((128, GROUPS, 128), np.float32)
    for g in range(GROUPS):
        maskF[p, g, (p % TG) * GROUPS + g] = 1.0
    maskF8 = np.zeros((128, GROUPS // 2, 64), np.float32)
    for g in range(GROUPS // 2):
        maskF8[p, g, (p % TG) * (GROUPS // 2) + g] = 1.0
    return mask, np.ascontiguousarray(mask.T), maskF, maskF8


def kernel(history, query, rms_weight):
    history = np.asarray(history, dtype=np.float32)
    query = np.asarray(query, dtype=np.float32)
    rms_weight = np.asarray(rms_weight, dtype=np.float32)
    assert history.shape == (S, B, T, D), history.shape

    nc = _get_program()
    mask, maskT, maskF, maskF8 = _make_masks()

    in_maps = []
    for c in range(N_CORES):
        b, h = c // 2, c % 2
        shard = np.ascontiguousarray(history[:, b, h * TC:(h + 1) * TC, :])
        in_maps.append({
            "hist": shard,
            "query": query,
            "rms_weight": rms_weight,
            "mask": mask,
            "maskT": maskT,
            "maskF": maskF,
            "maskF8": maskF8,
        })

    res = bass_utils.run_bass_kernel_spmd(nc, in_maps, list(range(N_CORES)))

    out = np.empty((B, T, D), dtype=np.float32)
    for c in range(N_CORES):
        b, h = c // 2, c % 2
        out[b, h * TC:(h + 1) * TC, :] = res.results[c]["out"]
    return out
